# revision 18
# baseline (speedup 1.0000x reference)
"""KBLN scorer kernel for 8 TRN2 NeuronCores.

out[b,e] = sum_f w[b,f] * exp(-(head_lit[b,f] - c[f] - lit[e,f])^2 / var[f])

Entities are sharded 8 ways.  Per feature f the Gaussian is expanded in a
cosine series of theta = pi*l/T_f (|theta| <= pi by construction):

    exp(-d^2/v) ~= sum_j c_j cos(j*pi/T * d),   d = a - l
    cos(j(ta-tl)) = cos(j ta)cos(j tl) + sin(j ta)sin(j tl)

so everything batch-dependent folds into host-computed matmul coefficients
and the device only needs, per entity chunk, rows spanning the harmonics
{cos(j tl), sin(j tl), j < JMAX}.  Those rows are built without any range
reduction from a single ACT Sin of the half angle phi = theta/2 (two
phases beta apart on the two partition halves), an ACT Square chain that
doubles angles (pre-affine re-literalizes each level), and DVE/Pool
elementwise products that fill odd harmonics.  Row pollution (affine and
cross terms) is absorbed into the coefficients by a host-side least
squares solve, and the DC term rides on the PSUM-evacuation bias.
"""

import numpy as np

import concourse.bass as bass
import concourse.tile as tile
from concourse import mybir
from concourse.bass_utils import run_bass_kernel_spmd
from concourse.tile import ScopedClock

E = 50000
F = 64
B = 64
NCORES = 8
E_SH = 6272          # padded shard: 8 * 6272 = 50176
E_PAD = E_SH * NCORES
SUB = 392            # one PSUM bank per matmul output
# small first chunk (fast pipeline fill) and small tail chunks (short drain)
CHUNKS = [392, 1568, 1568, 1568, 784, 392]
assert sum(CHUNKS) == E_SH and all(c % SUB == 0 for c in CHUNKS)
NCHUNK = len(CHUNKS)
CHUNK_OFF = [sum(CHUNKS[:i]) for i in range(NCHUNK)]
CHUNK_MAX = max(CHUNKS)

JMAX = 8             # harmonics 0..7
NT = 7               # content tiles: D1 D2 D3 P3 P5 P6 P7
NROW = 2 * NT + 1    # half-rows (top/bottom per tile) + DC

f32 = mybir.dt.float32
f32r = mybir.dt.float32r
ACTF = mybir.ActivationFunctionType


def _drain_and_barrier_split(self, tick_clock, wait_clock):
    # This walrus build accepts only one sync-wait per TPB_CTRL Drain;
    # spread the tail-drain waits across a chain of drains.
    drain_inst = self.nc.sync.drain()
    wait_clock.add_sem_waits(drain_inst.ins, ScopedClock({None: tick_clock.global_clock}))
    si = drain_inst.ins.sync_info
    waits = list(si.on_wait or [])
    if len(waits) > 1:
        si.on_wait = waits[:1]
        for w in waits[1:]:
            extra = self.nc.sync.drain()
            esi = extra.ins.sync_info
            if esi is None:
                from bass_rust import SyncInfo

                extra.ins.sync_info = SyncInfo(on_wait=[w], on_update=[])
            else:
                esi.on_wait = [w]
    self.nc.all_engine_barrier()
    popped = self.nc._tile_sem_poison_stack.pop()
    assert popped is self._sem_poison
    self.nc.clear_and_free_semaphores(list(self.sems.allocated().values()))
    self.nc.all_engine_barrier()


tile.TileContext._drain_and_barrier = _drain_and_barrier_split


def _split_excess_waits(nc, maxw=1):
    """This walrus build rejects instructions carrying more than one
    sync-wait. Hoist excess waits onto NOPs inserted just before the
    instruction on the same engine queue (same blocking semantics)."""
    from bass_rust import SyncInfo

    for f in nc.m.functions:
        for bb in f.blocks:
            new = []
            changed = False
            for inst in bb.instructions:
                si = inst.sync_info
                waits = list(si.on_wait) if si is not None and si.on_wait else []
                if len(waits) > maxw:
                    changed = True
                    extra, keep = waits[:-maxw], waits[-maxw:]
                    for i in range(0, len(extra), maxw):
                        nop = mybir.InstNoOp(
                            name=f"{inst.name}.w{i}",
                            engine=inst.engine,
                            ins=[],
                            outs=[],
                            sync_info=SyncInfo(
                                on_wait=extra[i : i + maxw], on_update=[]
                            ),
                        )
                        new.append(nop)
                    si.on_wait = keep
                new.append(inst)
            if changed:
                try:
                    bb.instructions[:] = new
                except TypeError:
                    bb.instructions = new


_NC_CACHE = None


def build_nc():
    global _NC_CACHE
    if _NC_CACHE is not None:
        return _NC_CACHE
    nc = bass.Bass(trn_type="TRN2")
    lit2 = nc.dram_tensor("lit2", [128, E_SH], f32, kind="ExternalInput")
    # scbi: col0 = pi/(2 T_f) (A1 scale), col1 = A1 bias (0 top, beta/2 bottom)
    # cols 2-5: square-chain affine constants 2.0, -0.5, 1.0, -1.0
    scbi = nc.dram_tensor("scbi", [128, 6], f32, kind="ExternalInput")
    lhsT = nc.dram_tensor("lhsT", [128, NT * B], f32r, kind="ExternalInput")
    dc = nc.dram_tensor("dc", [64, 1], f32, kind="ExternalInput")
    out = nc.dram_tensor("out", [B, E_SH], f32, kind="ExternalOutput")

    with tile.TileContext(nc) as tc:
        with (
            tc.tile_pool(name="singles", bufs=1) as singles,
            tc.tile_pool(name="lit", bufs=4) as litpool,
            tc.tile_pool(name="h", bufs=3) as hpool,
            tc.tile_pool(name="ps", bufs=8, space="PSUM") as pspool,
            tc.tile_pool(name="o", bufs=8) as opool,
        ):
            # ACT-critical inputs first so the square chain starts ASAP
            scbi_sb = singles.tile([128, 6], f32, tag="scbi")
            nc.sync.dma_start(out=scbi_sb, in_=scbi.ap())
            lit_tiles = []
            for k in range(NCHUNK):
                ksl = slice(CHUNK_OFF[k], CHUNK_OFF[k] + CHUNKS[k])
                lit_k = litpool.tile([128, CHUNKS[k]], f32, tag="lit", name=f"lit_{k}")
                nc.sync.dma_start(out=lit_k, in_=lit2.ap()[:, ksl])
                lit_tiles.append(lit_k)
            lhsT_sb = singles.tile([128, NT * B], f32r, tag="lhsT")
            nc.sync.dma_start(out=lhsT_sb, in_=lhsT.ap())
            dc_sb = singles.tile([64, 1], f32, tag="dc")
            nc.sync.dma_start(out=dc_sb, in_=dc.ap())

            c2 = scbi_sb[:, 2:3]
            cm05 = scbi_sb[:, 3:4]
            c1 = scbi_sb[:, 4:5]
            cm1 = scbi_sb[:, 5:6]

            pending = []  # psum tiles awaiting evacuation (software-pipelined)

            def flush_pending():
                for ps, osl in pending:
                    osb = opool.tile([B, SUB], f32, tag="o")
                    # evacuation adds the DC term via the per-partition scalar
                    nc.vector.tensor_scalar_add(osb, ps, dc_sb[:, 0:1])
                    nc.sync.dma_start(out=out.ap()[:, osl], in_=osb)
                pending.clear()

            for k in range(NCHUNK):
                lit_k = lit_tiles[k]
                CK = CHUNKS[k]

                def ht(name):
                    return hpool.tile([128, CK], f32r, tag=name, name=f"{name}_{k}")

                # A1 = sin(phi + [0; beta/2]), phi = pi*l/(2T)
                a1 = ht("a1")
                nc.scalar.activation(out=a1, in_=lit_k, func=ACTF.Sin,
                                     scale=scbi_sb[:, 0:1], bias=scbi_sb[:, 1:2])
                # D1 = (2 A1)^2 = 2(1 - cos th')
                d1 = ht("d1")
                nc.scalar.activation(out=d1, in_=a1, func=ACTF.Square, scale=c2)
                # D2 = (-0.5 D1 + 1)^2 = cos^2 = (1 + cos 2th')/2
                d2 = ht("d2")
                nc.scalar.activation(out=d2, in_=d1, func=ACTF.Square,
                                     scale=cm05, bias=c1)
                # D3 = (2 D2 - 1)^2 = (1 + cos 4th')/2
                d3 = ht("d3")
                nc.scalar.activation(out=d3, in_=d2, func=ACTF.Square,
                                     scale=c2, bias=cm1)
                # products fill remaining harmonics (coefs absorb scalings).
                # They are issued at PSUM-sub granularity so the stop-matmul
                # gate per sub is one small op, not a full-chunk Pool op.
                p3 = ht("p3")
                p5 = ht("p5")
                p6 = ht("p6")
                p7 = ht("p7")
                nsub_k = CK // SUB
                for j in range(nsub_k):
                    jsl = slice(j * SUB, (j + 1) * SUB)
                    nc.vector.tensor_mul(p3[:, jsl], d1[:, jsl], d2[:, jsl])
                for j in range(nsub_k):
                    jsl = slice(j * SUB, (j + 1) * SUB)
                    nc.vector.tensor_mul(p7[:, jsl], p3[:, jsl], d3[:, jsl])
                    if j % 2 == 0:
                        nc.vector.tensor_mul(p5[:, jsl], d1[:, jsl], d3[:, jsl])
                    else:
                        nc.gpsimd.tensor_mul(p5[:, jsl], d1[:, jsl], d3[:, jsl])
                    nc.gpsimd.tensor_mul(p6[:, jsl], d2[:, jsl], d3[:, jsl])

                # previous chunk's evacuations go behind this chunk's products
                # so they never head-of-line-block the DVE queue
                flush_pending()

                # accumulation ordered by expected tile readiness; stop on the
                # latest-ready tile so earlier matmuls never wait on it
                tiles = [d1, d2, d3, p3, p5, p6, p7]
                tidx = [0, 1, 2, 3, 4, 5, 6]  # lhsT column group per tile
                for j in range(CK // SUB):
                    jsl = slice(j * SUB, (j + 1) * SUB)
                    ps = pspool.tile([B, SUB], f32, tag="ps", name=f"ps_{k}_{j}")
                    for t, tl in enumerate(tiles):
                        nc.tensor.matmul(
                            ps,
                            lhsT=lhsT_sb[:, tidx[t] * B : (tidx[t] + 1) * B],
                            rhs=tl[:, jsl],
                            start=(t == 0),
                            stop=(t == NT - 1),
                        )
                    osl = slice(CHUNK_OFF[k] + j * SUB, CHUNK_OFF[k] + (j + 1) * SUB)
                    pending.append((ps, osl))
            flush_pending()
    _split_excess_waits(nc)
    _NC_CACHE = nc
    return nc


def _round_f32r(x):
    x32 = np.ascontiguousarray(x, dtype=np.float32)
    u = x32.view(np.uint32).copy()
    u = (u + 0x800) & 0xFFFF_F000
    return u.view(np.float32)


def _mul_rows(r1, r2):
    out = {}

    def add(j, ac, as_):
        if j < 0:
            j = -j
            as_ = -as_
        pc, ps = out.get(j, (0.0, 0.0))
        out[j] = (pc + ac, ps + as_)

    for j1, (ac1, as1) in r1.items():
        for j2, (ac2, as2) in r2.items():
            add(j1 + j2, 0.5 * (ac1 * ac2 - as1 * as2), 0.5 * (ac1 * as2 + as1 * ac2))
            add(j1 - j2, 0.5 * (ac1 * ac2 + as1 * as2), 0.5 * (-ac1 * as2 + as1 * ac2))
    return out


def _aff(r, s, b):
    out = {j: (s * ac, s * as_) for j, (ac, as_) in r.items()}
    pc, ps = out.get(0, (0.0, 0.0))
    out[0] = (pc + b, ps)
    return out


def _build_rows(ph):
    def cosrow(j, p):
        return {j: (np.cos(p), -np.sin(p))}

    D1 = _aff(cosrow(1, ph), -2.0, 2.0)
    D2 = _aff(cosrow(2, 2 * ph), 0.5, 0.5)
    D3 = _aff(cosrow(4, 4 * ph), 0.5, 0.5)
    P3 = _mul_rows(D1, D2)
    P5 = _mul_rows(D1, D3)
    P6 = _mul_rows(D2, D3)
    P7 = _mul_rows(P3, D3)
    return [D1, D2, D3, P3, P5, P6, P7]


_HMAX = 13  # representation space for row harmonics (P9 reaches 9)


def _to_vec(r):
    v = np.zeros(2 * _HMAX - 1)
    for j, (ac, as_) in r.items():
        assert 0 <= j < _HMAX
        if j == 0:
            v[0] += ac
        else:
            v[j] += ac
            v[_HMAX - 1 + j] += as_
    return v


def _host_prep(numerical_literals, c, var, nf_weights, head_ids, rel_ids):
    lit = np.asarray(numerical_literals, dtype=np.float64)
    c64 = np.asarray(c, dtype=np.float64)
    var64 = np.asarray(var, dtype=np.float64)
    w = np.asarray(nf_weights, dtype=np.float64)[np.asarray(rel_ids)]  # [B, F]
    a = lit[np.asarray(head_ids)] - c64                                # [B, F]

    Dmax = np.maximum(a.max(0) - lit.min(0), lit.max(0) - a.min(0))
    Labs = np.abs(lit).max(0)
    T = np.maximum((Dmax + 2.6 * np.sqrt(var64)) / 2.0, Labs + 1e-3)

    # beta maximizing min_j |sin(j beta)| over j=1..JMAX-1
    cands = np.linspace(0.05, 0.95, 901) * np.pi
    jj_all = np.arange(1, JMAX)
    scores = np.min(np.abs(np.sin(np.outer(cands, jj_all))), axis=1)
    beta = float(cands[np.argmax(scores)])

    rows_sym = []
    r0 = _build_rows(0.0)
    rb = _build_rows(beta)
    for i in range(NT):
        rows_sym.append(r0[i])
        rows_sym.append(rb[i])
    rows_sym.append({0: (1.0, 0.0)})
    M = np.stack([_to_vec(r) for r in rows_sym])        # [NROW, DIM]
    Mpinv = np.linalg.pinv(M.T, rcond=1e-8)             # [NROW, DIM]

    # least-squares cosine fit of exp(-d^2/v) on the actual d-range per f
    jj = np.arange(JMAX)
    cj = np.zeros((F, JMAX))
    for f in range(F):
        dg = np.linspace(0, Dmax[f] * 1.01, 600)
        gk = np.exp(-dg**2 / var64[f])
        A = np.cos(np.outer(dg, jj) * np.pi / T[f])
        cj[f], *_ = np.linalg.lstsq(A, gk, rcond=None)

    DIM = 2 * _HMAX - 1
    lhsT = np.zeros((128, NT, B), dtype=np.float64)
    dcv = np.zeros(B, dtype=np.float64)
    th_a = np.pi * a / T[None, :]                       # [B, F]
    for f in range(F):
        tgt = np.zeros((B, DIM))
        tgt[:, 0] = cj[f, 0]
        tgt[:, 1:JMAX] = cj[f, 1:] * np.cos(np.outer(th_a[:, f], jj[1:]))
        tgt[:, _HMAX:_HMAX - 1 + JMAX] = cj[f, 1:] * np.sin(np.outer(th_a[:, f], jj[1:]))
        x = Mpinv @ tgt.T                               # [NROW, B]
        xw = x * w[:, f][None, :]
        lhsT[f, :, :] = xw[0:2 * NT:2, :]
        lhsT[64 + f, :, :] = xw[1:2 * NT:2, :]
        dcv += xw[2 * NT, :]

    lhsT_r = _round_f32r(lhsT.reshape(128, NT * B))
    scbi = np.zeros((128, 6), dtype=np.float32)
    scbi[:F, 0] = (np.pi / (2 * T)).astype(np.float32)
    scbi[F:, 0] = scbi[:F, 0]
    scbi[F:, 1] = np.float32(beta / 2)
    scbi[:, 2:6] = np.array([2.0, -0.5, 1.0, -1.0], dtype=np.float32)
    dcarr = dcv.astype(np.float32).reshape(64, 1)

    litp = np.zeros((E_PAD, F), dtype=np.float32)
    litp[:E] = np.asarray(numerical_literals, dtype=np.float32)

    in_maps = []
    for i in range(NCORES):
        sh = litp[i * E_SH : (i + 1) * E_SH].T          # [F, E_SH]
        lit2 = np.ascontiguousarray(np.concatenate([sh, sh], axis=0))
        in_maps.append(
            {
                "lit2": lit2,
                "scbi": scbi,
                "lhsT": lhsT_r,
                "dc": dcarr,
            }
        )
    return in_maps


def kernel(numerical_literals, c, var, nf_weights, head_ids, rel_ids):
    nc = build_nc()
    in_maps = _host_prep(numerical_literals, c, var, nf_weights, head_ids, rel_ids)
    res = run_bass_kernel_spmd(nc, in_maps, core_ids=list(range(NCORES)))
    out = np.concatenate([res.results[i]["out"] for i in range(NCORES)], axis=1)
    return np.ascontiguousarray(out[:, :E])


# revision 20
# speedup vs baseline: 1.1204x; 1.1204x over previous
"""KBLN scorer kernel for 8 TRN2 NeuronCores.

out[b,e] = sum_f w[b,f] * exp(-(head_lit[b,f] - c[f] - lit[e,f])^2 / var[f])

Entities are sharded 8 ways.  Per feature f the Gaussian is expanded in a
cosine series of theta = pi*l/T_f (|theta| <= pi by construction):

    exp(-d^2/v) ~= sum_j c_j cos(j*pi/T * d),   d = a - l
    cos(j(ta-tl)) = cos(j ta)cos(j tl) + sin(j ta)sin(j tl)

so everything batch-dependent folds into host-computed matmul coefficients
and the device only needs, per entity chunk, rows spanning the harmonics
{cos(j tl), sin(j tl), j < JMAX}.  Those rows are built without any range
reduction from a single ACT Sin of the half angle phi = theta/2 (two
phases beta apart on the two partition halves), an ACT Square chain that
doubles angles (pre-affine re-literalizes each level), and DVE/Pool
elementwise products that fill odd harmonics.  Row pollution (affine and
cross terms) is absorbed into the coefficients by a host-side least
squares solve, and the DC term rides on the PSUM-evacuation bias.
"""

import numpy as np

import concourse.bass as bass
import concourse.tile as tile
from concourse import mybir
from concourse.bass_utils import run_bass_kernel_spmd
from concourse.tile import ScopedClock

E = 50000
F = 64
B = 64
NCORES = 8
E_SH = 6272          # padded shard: 8 * 6272 = 50176
E_PAD = E_SH * NCORES
SUB = 392            # one PSUM bank per matmul output
# small first chunk (fast pipeline fill) and small tail chunks (short drain)
CHUNKS = [392, 1568, 1568, 1568, 784, 392]
assert sum(CHUNKS) == E_SH and all(c % SUB == 0 for c in CHUNKS)
NCHUNK = len(CHUNKS)
CHUNK_OFF = [sum(CHUNKS[:i]) for i in range(NCHUNK)]
CHUNK_MAX = max(CHUNKS)

JMAX = 8             # harmonics 0..7
NT = 7               # content tiles: D1 D2 D3 P3 P5 P6 P7
NROW = 2 * NT + 1    # half-rows (top/bottom per tile) + DC

f32 = mybir.dt.float32
f32r = mybir.dt.float32r
ACTF = mybir.ActivationFunctionType


def _drain_and_barrier_split(self, tick_clock, wait_clock):
    # This walrus build accepts only one sync-wait per TPB_CTRL Drain;
    # spread the tail-drain waits across a chain of drains.
    drain_inst = self.nc.sync.drain()
    wait_clock.add_sem_waits(drain_inst.ins, ScopedClock({None: tick_clock.global_clock}))
    si = drain_inst.ins.sync_info
    waits = list(si.on_wait or [])
    if len(waits) > 1:
        si.on_wait = waits[:1]
        for w in waits[1:]:
            extra = self.nc.sync.drain()
            esi = extra.ins.sync_info
            if esi is None:
                from bass_rust import SyncInfo

                extra.ins.sync_info = SyncInfo(on_wait=[w], on_update=[])
            else:
                esi.on_wait = [w]
    self.nc.all_engine_barrier()
    popped = self.nc._tile_sem_poison_stack.pop()
    assert popped is self._sem_poison
    self.nc.clear_and_free_semaphores(list(self.sems.allocated().values()))
    self.nc.all_engine_barrier()


tile.TileContext._drain_and_barrier = _drain_and_barrier_split


def _split_excess_waits(nc, maxw=1):
    """This walrus build rejects instructions carrying more than one
    sync-wait. Hoist excess waits onto NOPs inserted just before the
    instruction on the same engine queue (same blocking semantics)."""
    from bass_rust import SyncInfo

    for f in nc.m.functions:
        for bb in f.blocks:
            new = []
            changed = False
            for inst in bb.instructions:
                si = inst.sync_info
                waits = list(si.on_wait) if si is not None and si.on_wait else []
                if len(waits) > maxw:
                    changed = True
                    extra, keep = waits[:-maxw], waits[-maxw:]
                    for i in range(0, len(extra), maxw):
                        nop = mybir.InstNoOp(
                            name=f"{inst.name}.w{i}",
                            engine=inst.engine,
                            ins=[],
                            outs=[],
                            sync_info=SyncInfo(
                                on_wait=extra[i : i + maxw], on_update=[]
                            ),
                        )
                        new.append(nop)
                    si.on_wait = keep
                new.append(inst)
            if changed:
                try:
                    bb.instructions[:] = new
                except TypeError:
                    bb.instructions = new


_NC_CACHE = None


def build_nc():
    global _NC_CACHE
    if _NC_CACHE is not None:
        return _NC_CACHE
    nc = bass.Bass(trn_type="TRN2")
    lit2 = nc.dram_tensor("lit2", [128, E_SH], f32, kind="ExternalInput")
    # scbi: col0 = pi/(2 T_f) (A1 scale), col1 = A1 bias (0 top, beta/2 bottom)
    # cols 2-5: square-chain affine constants 2.0, -0.5, 1.0, -1.0
    scbi = nc.dram_tensor("scbi", [128, 6], f32, kind="ExternalInput")
    lhsT = nc.dram_tensor("lhsT", [128, NT * B], f32r, kind="ExternalInput")
    dc = nc.dram_tensor("dc", [64, 1], f32, kind="ExternalInput")
    out = nc.dram_tensor("out", [B, E_SH], f32, kind="ExternalOutput")

    with tile.TileContext(nc) as tc:
        with (
            tc.tile_pool(name="singles", bufs=1) as singles,
            tc.tile_pool(name="lit", bufs=4) as litpool,
            tc.tile_pool(name="h", bufs=3) as hpool,
            tc.tile_pool(name="ps", bufs=8, space="PSUM") as pspool,
            tc.tile_pool(name="o", bufs=8) as opool,
        ):
            # ACT-critical inputs first so the square chain starts ASAP
            scbi_sb = singles.tile([128, 6], f32, tag="scbi")
            nc.sync.dma_start(out=scbi_sb, in_=scbi.ap())
            lit_tiles = []
            for k in range(NCHUNK):
                ksl = slice(CHUNK_OFF[k], CHUNK_OFF[k] + CHUNKS[k])
                lit_k = litpool.tile([128, CHUNKS[k]], f32, tag="lit", name=f"lit_{k}")
                nc.sync.dma_start(out=lit_k, in_=lit2.ap()[:, ksl])
                lit_tiles.append(lit_k)
            lhsT_sb = singles.tile([128, NT * B], f32r, tag="lhsT")
            nc.sync.dma_start(out=lhsT_sb, in_=lhsT.ap())
            dc_sb = singles.tile([64, 1], f32, tag="dc")
            nc.sync.dma_start(out=dc_sb, in_=dc.ap())

            warm = singles.tile([128, 1], f32, tag="warm")
            nc.vector.memset(warm, 0.0)
            nc.scalar.activation(out=warm, in_=warm, func=ACTF.Sin)

            c2 = scbi_sb[:, 2:3]
            cm05 = scbi_sb[:, 3:4]
            c1 = scbi_sb[:, 4:5]
            cm1 = scbi_sb[:, 5:6]

            pending = []  # psum tiles awaiting evacuation (software-pipelined)

            def flush_pending(on_act=False):
                for ps, base in pending:
                    osb = opool.tile([B, SUB], f32, tag="o")
                    # evacuation adds the DC term via the per-partition bias;
                    # late chunks evacuate on ACT, which is idle by then
                    if on_act:
                        nc.scalar.activation(out=osb, in_=ps, func=ACTF.Identity,
                                             bias=dc_sb[:, 0:1])
                    else:
                        nc.vector.tensor_scalar_add(osb, ps, dc_sb[:, 0:1])
                    nc.sync.dma_start(out=out.ap()[:, base : base + SUB], in_=osb)
                pending.clear()

            for k in range(NCHUNK):
                lit_k = lit_tiles[k]
                CK = CHUNKS[k]

                def ht(name):
                    return hpool.tile([128, CK], f32r, tag=name, name=f"{name}_{k}")

                # A1 = sin(phi + [0; beta/2]), phi = pi*l/(2T)
                a1 = ht("a1")
                nc.scalar.activation(out=a1, in_=lit_k, func=ACTF.Sin,
                                     scale=scbi_sb[:, 0:1], bias=scbi_sb[:, 1:2])
                # D1 = (2 A1)^2 = 2(1 - cos th')
                d1 = ht("d1")
                nc.scalar.activation(out=d1, in_=a1, func=ACTF.Square, scale=c2)
                # D2 = (-0.5 D1 + 1)^2 = cos^2 = (1 + cos 2th')/2
                d2 = ht("d2")
                nc.scalar.activation(out=d2, in_=d1, func=ACTF.Square,
                                     scale=cm05, bias=c1)
                # D3 = (2 D2 - 1)^2 = (1 + cos 4th')/2
                d3 = ht("d3")
                nc.scalar.activation(out=d3, in_=d2, func=ACTF.Square,
                                     scale=c2, bias=cm1)
                # products fill remaining harmonics (coefs absorb scalings).
                # They are issued at PSUM-sub granularity so the stop-matmul
                # gate per sub is one small op, not a full-chunk Pool op.
                p3 = ht("p3")
                p5 = ht("p5")
                p6 = ht("p6")
                p7 = ht("p7")
                nsub_k = CK // SUB
                for j in range(nsub_k):
                    jsl = slice(j * SUB, (j + 1) * SUB)
                    nc.vector.tensor_mul(p3[:, jsl], d1[:, jsl], d2[:, jsl])
                for j in range(nsub_k):
                    jsl = slice(j * SUB, (j + 1) * SUB)
                    nc.vector.tensor_mul(p7[:, jsl], p3[:, jsl], d3[:, jsl])
                    if j % 2 == 0:
                        nc.vector.tensor_mul(p5[:, jsl], d1[:, jsl], d3[:, jsl])
                    else:
                        nc.gpsimd.tensor_mul(p5[:, jsl], d1[:, jsl], d3[:, jsl])
                    nc.gpsimd.tensor_mul(p6[:, jsl], d2[:, jsl], d3[:, jsl])

                # previous chunk's evacuations go behind this chunk's products
                # so they never head-of-line-block the DVE queue
                flush_pending(on_act=(k >= 4))

                # accumulation ordered by expected tile readiness; stop on the
                # latest-ready tile so earlier matmuls never wait on it
                tiles = [d1, d2, d3, p3, p5, p6, p7]
                tidx = [0, 1, 2, 3, 4, 5, 6]  # lhsT column group per tile
                for j in range(CK // SUB):
                    jsl = slice(j * SUB, (j + 1) * SUB)
                    ps = pspool.tile([B, SUB], f32, tag="ps", name=f"ps_{k}_{j}")
                    for t, tl in enumerate(tiles):
                        nc.tensor.matmul(
                            ps,
                            lhsT=lhsT_sb[:, tidx[t] * B : (tidx[t] + 1) * B],
                            rhs=tl[:, jsl],
                            start=(t == 0),
                            stop=(t == NT - 1),
                        )
                    pending.append((ps, CHUNK_OFF[k] + j * SUB))
            flush_pending(on_act=True)
    _split_excess_waits(nc)
    _NC_CACHE = nc
    return nc


def _round_f32r(x):
    x32 = np.ascontiguousarray(x, dtype=np.float32)
    u = x32.view(np.uint32).copy()
    u = (u + 0x800) & 0xFFFF_F000
    return u.view(np.float32)


def _mul_rows(r1, r2):
    out = {}

    def add(j, ac, as_):
        if j < 0:
            j = -j
            as_ = -as_
        pc, ps = out.get(j, (0.0, 0.0))
        out[j] = (pc + ac, ps + as_)

    for j1, (ac1, as1) in r1.items():
        for j2, (ac2, as2) in r2.items():
            add(j1 + j2, 0.5 * (ac1 * ac2 - as1 * as2), 0.5 * (ac1 * as2 + as1 * ac2))
            add(j1 - j2, 0.5 * (ac1 * ac2 + as1 * as2), 0.5 * (-ac1 * as2 + as1 * ac2))
    return out


def _aff(r, s, b):
    out = {j: (s * ac, s * as_) for j, (ac, as_) in r.items()}
    pc, ps = out.get(0, (0.0, 0.0))
    out[0] = (pc + b, ps)
    return out


def _build_rows(ph):
    def cosrow(j, p):
        return {j: (np.cos(p), -np.sin(p))}

    D1 = _aff(cosrow(1, ph), -2.0, 2.0)
    D2 = _aff(cosrow(2, 2 * ph), 0.5, 0.5)
    D3 = _aff(cosrow(4, 4 * ph), 0.5, 0.5)
    P3 = _mul_rows(D1, D2)
    P5 = _mul_rows(D1, D3)
    P6 = _mul_rows(D2, D3)
    P7 = _mul_rows(P3, D3)
    return [D1, D2, D3, P3, P5, P6, P7]


_HMAX = 13  # representation space for row harmonics (P9 reaches 9)


def _to_vec(r):
    v = np.zeros(2 * _HMAX - 1)
    for j, (ac, as_) in r.items():
        assert 0 <= j < _HMAX
        if j == 0:
            v[0] += ac
        else:
            v[j] += ac
            v[_HMAX - 1 + j] += as_
    return v


def _host_prep(numerical_literals, c, var, nf_weights, head_ids, rel_ids):
    lit = np.asarray(numerical_literals, dtype=np.float64)
    c64 = np.asarray(c, dtype=np.float64)
    var64 = np.asarray(var, dtype=np.float64)
    w = np.asarray(nf_weights, dtype=np.float64)[np.asarray(rel_ids)]  # [B, F]
    a = lit[np.asarray(head_ids)] - c64                                # [B, F]

    Dmax = np.maximum(a.max(0) - lit.min(0), lit.max(0) - a.min(0))
    Labs = np.abs(lit).max(0)
    T = np.maximum((Dmax + 2.6 * np.sqrt(var64)) / 2.0, Labs + 1e-3)

    # beta maximizing min_j |sin(j beta)| over j=1..JMAX-1
    cands = np.linspace(0.05, 0.95, 901) * np.pi
    jj_all = np.arange(1, JMAX)
    scores = np.min(np.abs(np.sin(np.outer(cands, jj_all))), axis=1)
    beta = float(cands[np.argmax(scores)])

    rows_sym = []
    r0 = _build_rows(0.0)
    rb = _build_rows(beta)
    for i in range(NT):
        rows_sym.append(r0[i])
        rows_sym.append(rb[i])
    rows_sym.append({0: (1.0, 0.0)})
    M = np.stack([_to_vec(r) for r in rows_sym])        # [NROW, DIM]
    Mpinv = np.linalg.pinv(M.T, rcond=1e-8)             # [NROW, DIM]

    # least-squares cosine fit of exp(-d^2/v) on the actual d-range per f
    jj = np.arange(JMAX)
    cj = np.zeros((F, JMAX))
    for f in range(F):
        dg = np.linspace(0, Dmax[f] * 1.01, 600)
        gk = np.exp(-dg**2 / var64[f])
        A = np.cos(np.outer(dg, jj) * np.pi / T[f])
        cj[f], *_ = np.linalg.lstsq(A, gk, rcond=None)

    DIM = 2 * _HMAX - 1
    lhsT = np.zeros((128, NT, B), dtype=np.float64)
    dcv = np.zeros(B, dtype=np.float64)
    th_a = np.pi * a / T[None, :]                       # [B, F]
    for f in range(F):
        tgt = np.zeros((B, DIM))
        tgt[:, 0] = cj[f, 0]
        tgt[:, 1:JMAX] = cj[f, 1:] * np.cos(np.outer(th_a[:, f], jj[1:]))
        tgt[:, _HMAX:_HMAX - 1 + JMAX] = cj[f, 1:] * np.sin(np.outer(th_a[:, f], jj[1:]))
        x = Mpinv @ tgt.T                               # [NROW, B]
        xw = x * w[:, f][None, :]
        lhsT[f, :, :] = xw[0:2 * NT:2, :]
        lhsT[64 + f, :, :] = xw[1:2 * NT:2, :]
        dcv += xw[2 * NT, :]

    lhsT_r = _round_f32r(lhsT.reshape(128, NT * B))
    scbi = np.zeros((128, 6), dtype=np.float32)
    scbi[:F, 0] = (np.pi / (2 * T)).astype(np.float32)
    scbi[F:, 0] = scbi[:F, 0]
    scbi[F:, 1] = np.float32(beta / 2)
    scbi[:, 2:6] = np.array([2.0, -0.5, 1.0, -1.0], dtype=np.float32)
    dcarr = dcv.astype(np.float32).reshape(64, 1)

    litp = np.zeros((E_PAD, F), dtype=np.float32)
    litp[:E] = np.asarray(numerical_literals, dtype=np.float32)

    in_maps = []
    for i in range(NCORES):
        sh = litp[i * E_SH : (i + 1) * E_SH].T          # [F, E_SH]
        lit2 = np.ascontiguousarray(np.concatenate([sh, sh], axis=0))
        in_maps.append(
            {
                "lit2": lit2,
                "scbi": scbi,
                "lhsT": lhsT_r,
                "dc": dcarr,
            }
        )
    return in_maps


def kernel(numerical_literals, c, var, nf_weights, head_ids, rel_ids):
    nc = build_nc()
    in_maps = _host_prep(numerical_literals, c, var, nf_weights, head_ids, rel_ids)
    res = run_bass_kernel_spmd(nc, in_maps, core_ids=list(range(NCORES)))
    out = np.concatenate([res.results[i]["out"] for i in range(NCORES)], axis=1)
    return np.ascontiguousarray(out[:, :E])
